# revision 13
# baseline (speedup 1.0000x reference)
"""Trainium2 Bass kernel for GroupNorm -> self-attention -> proj + residual.

v5: fp16 matmuls for qkv/scores/proj; fp8 DoubleRow for the v-projection,
attn.V and colsum.  Colsum runs as a transient PSUM group at the end of
the scores stage so the big-PSUM pool gets 3 rotating buffers (6 banks).

Reference computation (per image, b=32 total, data-parallel over 8 cores):
    xn    = GroupNorm(x, 8 groups, affine)              [c=256, n=1024]
    q,k   = W_{q,k} @ xn + b_{q,k}      (fp16 matmul, fp16 drain)
    v     = W_v @ xn                    (fp8 DR matmul, fp8 drain)
    st    = k^T q                       (fp16, scores transposed [nk, nq])
    est   = exp(st/16 - C)              (C = 2.7 global shift, softmax-invariant)
    den   = colsum(est)                 (ones-matmul DR, transient PSUM)
    outT  = (v^T est) / den   [c, nq]   (fp8 DR)
    fin   = out_w @ outT + (out_w @ v_b + out_b) + xn   (fp16)

Design notes (HW-measured):
  - fp16 matmul (K=128, 512-wide) ~295ns HW; DR fp8 (K=256, 512-wide)
    ~255ns.  fp8 q/k for a DR scores matmul was measured at 2.4e-2 total
    rel err vs the 2e-2 gate (HW matches RTNE host emulation almost
    exactly; the fp8 est+v floor is ~1.2e-2) — so scores stay fp16.
  - est = exp(score/16 - C) in fp8e4 on ACT; C=2.7 keeps max est ~145,
    under the TRN e4m3 240 cap.
  - Softmax denominator folds into the outT drain; outT/out_w stay fp16.
  - Every PSUM tile has exactly ONE drain op (ACT or DVE) so PSUM buffers
    recycle fast; SBUF-only work (GroupNorm apply, residual prep) runs on
    the otherwise idle GPSIMD/Pool engine (which cannot touch PSUM).
  - Consecutive matmuls share their stationary operand where possible
    (loop orders chosen so lhsT repeats back-to-back).

Stages: S0 dma+GroupNorm, S1 qkv/vt, S2 scores+exp+colsum, S3 recip+attnv,
S4 proj+out-dma; emission is skewed so every engine always has runnable
work.  The reps timing loop runs `unroll` pipelined copies per iteration to
amortize For_i's per-iteration all-engine barrier and pipeline fill/drain.
"""

import numpy as np
import ml_dtypes
from contextlib import ExitStack

import concourse.bass as bass
import concourse.tile as tile
import concourse.mybir as mybir
from concourse import bacc
from concourse.bass import ts
from concourse.bass_utils import run_bass_kernel_spmd

P = 128
N_CORES = 8
B, C, H, W = 32, 256, 32, 32
N = H * W                      # 1024 pixels
IMGS = B // N_CORES            # 4 images per core
NH = C // P                    # 2 channel halves
NT = N // P                    # 8 pixel tiles
GROUPS = 8
EPS = 1e-5
F32 = mybir.dt.float32
F16 = mybir.dt.float16
F8 = mybir.dt.float8e4
AF = mybir.ActivationFunctionType
OP = mybir.AluOpType
DR = mybir.MatmulPerfMode.DoubleRow
CHUNK = 512                    # matmul moving free dim (one PSUM bank)
NCH = N // CHUNK               # 2 chunks
C_SHIFT = 2.7                  # global exp shift (softmax-invariant)

PHASE_OF = {}


class _phase:
    """Records which instructions each phase emits (for trace attribution)."""

    def __init__(self, nc, name):
        self.nc, self.name = nc, name

    def __enter__(self):
        self.before = set(self.nc.inst_map)
        return self

    def __exit__(self, *a):
        for n in set(self.nc.inst_map) - self.before:
            PHASE_OF[n] = self.name


class _State:
    """Per-image tiles passed between pipeline stages."""

    def __init__(self):
        self.xn16 = {}
        self.xn8 = {}
        self.xnfb = {}
        self.qk = {}
        self.vt = {}
        self.est = {}
        self.cs = {}
        self.recip = {}
        self.outt = {}


def _emit(ctx: ExitStack, tc: tile.TileContext, t: dict, reps: int = 1,
          unroll: int = 1):
    nc = tc.nc

    singles = ctx.enter_context(tc.tile_pool(name="singles", bufs=1))
    p_x = ctx.enter_context(tc.tile_pool(name="p_x", bufs=2))
    p_stats = ctx.enter_context(tc.tile_pool(name="p_stats", bufs=4))
    p_xn16 = ctx.enter_context(tc.tile_pool(name="p_xn16", bufs=3))
    p_xn8 = ctx.enter_context(tc.tile_pool(name="p_xn8", bufs=3))
    p_xnfb = ctx.enter_context(tc.tile_pool(name="p_xnfb", bufs=5))
    p_qk = ctx.enter_context(tc.tile_pool(name="p_qk", bufs=3))
    p_vt = ctx.enter_context(tc.tile_pool(name="p_vt", bufs=4))
    p_est = ctx.enter_context(tc.tile_pool(name="p_est", bufs=3))
    p_recip = ctx.enter_context(tc.tile_pool(name="p_recip", bufs=2))
    p_outt = ctx.enter_context(tc.tile_pool(name="p_outt", bufs=3))
    p_fin = ctx.enter_context(tc.tile_pool(name="p_fin", bufs=4))
    ps_big = ctx.enter_context(tc.tile_pool(name="ps_big", bufs=3, space="PSUM"))
    ps_sm = ctx.enter_context(tc.tile_pool(name="ps_sm", bufs=1, space="PSUM"))

    # ---- load constants / weights into SBUF once ----
    s_wqk16 = singles.tile([P, NH, 512], F16)
    nc.sync.dma_start(s_wqk16[:], t["wqk16"].rearrange("h p o -> p h o"))
    s_wv8 = singles.tile([P, NH, C], F8)
    nc.sync.dma_start(s_wv8[:], t["wv8"].rearrange("h p o -> p h o"))
    s_woT = singles.tile([P, NH, C], F16)
    nc.sync.dma_start(s_woT[:], t["woT"].rearrange("h p o -> p h o"))
    s_bqk = singles.tile([P, 4], F32)
    nc.sync.dma_start(s_bqk[:], t["bqk"].rearrange("j p -> p j"))
    s_gnw = singles.tile([P, NH], F32)
    nc.sync.dma_start(s_gnw[:], t["gnw"].rearrange("h p -> p h"))
    s_gnb = singles.tile([P, NH], F32)
    nc.sync.dma_start(s_gnb[:], t["gnb"].rearrange("h p -> p h"))
    s_fbv = singles.tile([P, NH], F32)      # out_w @ v_b + out_b (residual add)
    nc.sync.dma_start(s_fbv[:], t["fbv"].rearrange("h p -> p h"))
    s_ind = singles.tile([P, NH, GROUPS], F32)
    nc.sync.dma_start(s_ind[:], t["ind"].rearrange("h p g -> p h g"))
    s_indT = singles.tile([GROUPS, NH, P], F32)
    nc.sync.dma_start(s_indT[:], t["indT"])
    s_ones = singles.tile([P, NH, P], F8)
    nc.vector.memset(s_ones[:], 1.0)
    s_negC = singles.tile([P, 1], F32)
    nc.vector.memset(s_negC[:], -C_SHIFT)

    # PE warmup: dense dummy matmuls during the GroupNorm head so the HAM
    # clock-gate reaches 8/8 before the real matmuls start (HW-only effect).
    ps_w = ps_big.tile([P, N], F32, tag="big")
    for _ in range(10):
        nc.tensor.matmul(ps_w[:, 0:CHUNK], s_wqk16[:, 0, 0:P],
                         s_wqk16[:, 1, 0:CHUNK], start=True, stop=True)
    w_sink = p_stats.tile([1, 1], F32, tag="wsink")
    nc.vector.tensor_copy(w_sink[:], ps_w[0:1, 0:1])

    x_ap = t["x"]       # [IMGS, NH, P, N] f16
    out_ap = t["out"]   # [IMGS, NH, P, N] f16

    if reps > 1:
        loop = ctx.enter_context(  # noqa: F841 (timing loop)
            tc.For_i(0, reps // unroll, 1,
                     hint_engines=(mybir.EngineType.PE,)))

    st = _State()

    def s0_gn(vimg):
        img = vimg % IMGS
        x_t = p_x.tile([P, NH, N], F16, tag="x")
        xn16 = st.xn16[vimg] = p_xn16.tile([P, NH, N], F16, tag="xn16",
                                           name=f"xn16_{vimg}")
        xn8 = st.xn8[vimg] = p_xn8.tile([P, NH, N], F8, tag="xn8",
                                        name=f"xn8_{vimg}")
        xnfb = st.xnfb[vimg] = p_xnfb.tile([P, NH, N], F16, tag="xnfb",
                                           name=f"xnfb_{vimg}")
        nc.sync.dma_start(x_t[:], x_ap[img].rearrange("h p n -> p h n"))
        for h in range(NH):
            # per-channel mean / E[x^2] via bn_stats (free dim cap 512)
            st6 = p_stats.tile([P, 2, 6], F32, tag="st6")
            xv = x_t[:, h].rearrange("p (s f) -> p s f", f=512)
            for s in range(2):
                nc.vector.bn_stats(out=st6[:, s, :], in_=xv[:, s, :])
            mv = p_stats.tile([P, 2], F32, tag="mv")
            nc.vector.bn_aggr(out=mv[:], in_=st6[:])
            mm = p_stats.tile([P, 2], F32, tag="mm")  # (mean, E[x^2])
            nc.vector.tensor_copy(mm[:, 0:1], mv[:, 0:1])
            nc.vector.tensor_tensor(mm[:, 1:2], mv[:, 0:1], mv[:, 0:1], OP.mult)
            nc.vector.tensor_tensor(mm[:, 1:2], mm[:, 1:2], mv[:, 1:2], OP.add)

            # this half's 4 group stats: [4, 2] = ind_h.T @ mm
            psg = ps_sm.tile([4, 2], F32, tag="sm")
            nc.tensor.matmul(psg[:], s_ind[:, h, :4], mm[:],
                             start=True, stop=True)
            grp = p_stats.tile([4, 2], F32, tag="grp")  # (mu, rstd)
            nc.vector.tensor_copy(grp[:, 0:1], psg[:, 0:1])
            nc.vector.tensor_copy(grp[:, 1:2], psg[:, 1:2])
            v = p_stats.tile([4, 3], F32, tag="musq")  # var+eps, s, t
            nc.vector.tensor_tensor(v[:, 1:2], grp[:, 0:1], grp[:, 0:1], OP.mult)
            nc.vector.tensor_tensor(v[:, 0:1], grp[:, 1:2], v[:, 1:2], OP.subtract)
            nc.vector.tensor_scalar(out=v[:, 0:1], in0=v[:, 0:1], scalar1=EPS,
                                    scalar2=None, op0=OP.add)
            # rstd = 1/sqrt(v) by Newton on sqrt from s0=1 (group var ~ 1),
            # all on DVE — keeps ACT's table set pinned to exp.
            nc.vector.tensor_scalar(out=v[:, 1:2], in0=v[:, 0:1], scalar1=1.0,
                                    scalar2=0.5, op0=OP.add, op1=OP.mult)
            for _ in range(2):
                nc.vector.reciprocal(v[:, 2:3], v[:, 1:2])
                nc.vector.tensor_tensor(v[:, 2:3], v[:, 0:1], v[:, 2:3], OP.mult)
                nc.vector.tensor_tensor(v[:, 1:2], v[:, 1:2], v[:, 2:3], OP.add)
                nc.vector.tensor_scalar(out=v[:, 1:2], in0=v[:, 1:2],
                                        scalar1=0.5, scalar2=None, op0=OP.mult)
            nc.vector.reciprocal(grp[:, 1:2], v[:, 1:2])

            # broadcast 4 group (mu, rstd) to this half's 128 channels
            psb = ps_sm.tile([P, 2], F32, tag="sm")
            nc.tensor.matmul(psb[:], s_indT[:4, h, :], grp[:],
                             start=True, stop=True)
            ab = p_stats.tile([P, 2], F32, tag="ab")  # a, b
            a = ab[:, 0:1]
            nc.vector.tensor_tensor(a, psb[:, 1:2], s_gnw[:, h:h + 1], OP.mult)
            mua = ab[:, 1:2]
            nc.vector.tensor_tensor(mua, psb[:, 0:1], a, OP.mult)
            nc.vector.tensor_tensor(mua, s_gnb[:, h:h + 1], mua, OP.subtract)

            # xn16 = f16(x*a+b); xn8 = f8 same; xnfb = f16(xn16 + fb)
            # (Pool: SBUF-only ops; Pool's fp8 rounding verified == DVE/ACT)
            nc.gpsimd.tensor_scalar(out=xn16[:, h], in0=x_t[:, h],
                                    scalar1=ab[:, 0:1], scalar2=ab[:, 1:2],
                                    op0=OP.mult, op1=OP.add)
            nc.gpsimd.tensor_scalar(out=xn8[:, h], in0=x_t[:, h],
                                    scalar1=ab[:, 0:1], scalar2=ab[:, 1:2],
                                    op0=OP.mult, op1=OP.add)
            nc.gpsimd.tensor_scalar(out=xnfb[:, h], in0=xn16[:, h],
                                    scalar1=s_fbv[:, h:h + 1], scalar2=None,
                                    op0=OP.add)

    def s1_qkv(vimg):
        xn16, xn8 = st.xn16[vimg], st.xn8[vimg]
        # fp16 q,k (biased in the drain): slots (q0,q1,k0,k1).  Scores must
        # stay fp16: fp8 q/k measured 2.0-2.4e-2 total rel err vs the 2e-2
        # gate (fp8 est+v alone already costs ~1.2e-2).
        qk = st.qk[vimg] = p_qk.tile([P, 4, N], F16, tag="qk",
                                     name=f"qk_{vimg}")
        for j in range(4):
            ps = ps_big.tile([P, N], F32, tag="big")
            for h in range(NH):       # h outer: consecutive MMs share lhsT
                for ch in range(NCH):
                    nc.tensor.matmul(ps[:, ts(ch, CHUNK)],
                                     s_wqk16[:, h, ts(j, P)],
                                     xn16[:, h, ts(ch, CHUNK)],
                                     start=(h == 0), stop=(h == NH - 1))
            if j < 2:
                nc.scalar.activation(out=qk[:, j], in_=ps[:], func=AF.Identity,
                                     bias=s_bqk[:, j:j + 1])
            else:
                nc.vector.tensor_scalar(out=qk[:, j], in0=ps[:],
                                        scalar1=s_bqk[:, j:j + 1], scalar2=None,
                                        op0=OP.add)

        # vT in [n, c] layout via fp8 DR (K=256 over the half pair)
        vt = st.vt[vimg] = p_vt.tile([P, NT, C], F8, tag="vt",
                                     name=f"vt_{vimg}")
        for tg in range(2):
            ps = ps_big.tile([P, N], F32, tag="big")
            for tl in range(4):
                tt = 4 * tg + tl
                nc.tensor.matmul(ps[:, ts(tl, C)],
                                 xn8[:, 0:NH, ts(tt, P)], s_wv8[:, 0:NH, :],
                                 start=True, stop=True, perf_mode=DR)
            nc.scalar.activation(out=vt[:, 4 * tg:4 * tg + 4], in_=ps[:],
                                 func=AF.Identity)

    def s2_scores(vimg):
        qk = st.qk[vimg]
        est = st.est[vimg] = p_est.tile([P, NT, N], F8, tag="est",
                                        name=f"est_{vimg}")
        for tt in range(NT):
            ps = ps_big.tile([P, N], F32, tag="big")
            for h in range(NH):       # h outer: consecutive MMs share lhsT
                for ch in range(NCH):
                    nc.tensor.matmul(ps[:, ts(ch, CHUNK)],
                                     qk[:, 2 + h, ts(tt, P)],
                                     qk[:, 0 + h, ts(ch, CHUNK)],
                                     start=(h == 0), stop=(h == NH - 1))
            nc.scalar.activation(out=est[:, tt], in_=ps[:], func=AF.Exp,
                                 bias=s_negC[:], scale=1.0 / 16.0)
        # colsum of est (ones-matmul DR, one shared stationary), transient
        # PSUM group drained by the recip in s3
        cs = st.cs[vimg] = ps_big.tile([P, N], F32, tag="big",
                                       name=f"cs_{vimg}")
        for i in range(NT // 2):
            for ch in range(NCH):
                nc.tensor.matmul(cs[:, ts(ch, CHUNK)], s_ones[:],
                                 est[:, 2 * i:2 * i + 2, ts(ch, CHUNK)],
                                 start=(i == 0), stop=(i == NT // 2 - 1),
                                 perf_mode=DR)

    def s3_attnv(vimg):
        vt, est, cs = st.vt[vimg], st.est[vimg], st.cs[vimg]
        recip = st.recip[vimg] = p_recip.tile([P, N], F32, tag="recip",
                                              name=f"recip_{vimg}")
        nc.vector.reciprocal(recip[:], cs[:])
        outt = st.outt[vimg] = p_outt.tile([P, NH, N], F16, tag="outt",
                                           name=f"outt_{vimg}")
        for m in range(NH):
            ps = ps_big.tile([P, N], F32, tag="big")
            for i in range(NT // 2):
                for ch in range(NCH):
                    nc.tensor.matmul(ps[:, ts(ch, CHUNK)],
                                     vt[:, 2 * i:2 * i + 2, ts(m, P)],
                                     est[:, 2 * i:2 * i + 2, ts(ch, CHUNK)],
                                     start=(i == 0), stop=(i == NT // 2 - 1),
                                     perf_mode=DR)
            # normalize during copyback (recip commutes with the c-contraction)
            nc.vector.tensor_tensor(outt[:, m], ps[:], recip[:], OP.mult)

    def s4_proj(vimg):
        img = vimg % IMGS
        outt, xnfb = st.outt[vimg], st.xnfb[vimg]
        for m in range(NH):
            ps = ps_big.tile([P, N], F32, tag="big")
            for h in range(NH):       # h outer: consecutive MMs share lhsT
                for ch in range(NCH):
                    nc.tensor.matmul(ps[:, ts(ch, CHUNK)],
                                     s_woT[:, h, ts(m, P)],
                                     outt[:, h, ts(ch, CHUNK)],
                                     start=(h == 0), stop=(h == NH - 1))
            fin = p_fin.tile([P, N], F16, tag="fin")
            nc.vector.tensor_tensor(fin[:], ps[:], xnfb[:, m], OP.add)
            nc.sync.dma_start(out_ap[img, m], fin[:])

    def _tap(vimg, stage_i):
        if vimg != 0 or "d_xn16" not in t:
            return
        if stage_i == 0:
            nc.sync.dma_start(t["d_xn16"], st.xn16[0][:])
            nc.sync.dma_start(t["d_xnfb"], st.xnfb[0][:])
        elif stage_i == 1:
            nc.sync.dma_start(t["d_qk"], st.qk[0][:])
            nc.sync.dma_start(t["d_vt"], st.vt[0][:])
        elif stage_i == 2:
            nc.sync.dma_start(t["d_est"], st.est[0][:])
        elif stage_i == 3:
            nc.sync.dma_start(t["d_recip"], st.recip[0][:])
            nc.sync.dma_start(t["d_outt"], st.outt[0][:])

    stages = [s0_gn, s1_qkv, s2_scores, s3_attnv, s4_proj]
    names = ["gn", "qkv", "scores", "attnv", "proj"]
    NS = len(stages)
    VIMGS = IMGS * unroll
    for t_step in range(VIMGS + NS - 1):
        for s in range(NS - 1, -1, -1):     # older images' later stages first
            vimg = t_step - s
            if 0 <= vimg < VIMGS:
                with _phase(nc, names[s]):
                    stages[s](vimg)
                    _tap(vimg, s)


def _build(reps: int = 1, unroll: int = 1, taps: bool = False):
    nc = bacc.Bacc("TRN2", debug=False, num_devices=N_CORES)
    t = {}
    if taps:
        t["d_xn16"] = nc.dram_tensor("d_xn16", [P, NH, N], F16, kind="ExternalOutput").ap()
        t["d_xnfb"] = nc.dram_tensor("d_xnfb", [P, NH, N], F16, kind="ExternalOutput").ap()
        t["d_qk"] = nc.dram_tensor("d_qk", [P, 4, N], F16, kind="ExternalOutput").ap()
        t["d_vt"] = nc.dram_tensor("d_vt", [P, NT, C], F8, kind="ExternalOutput").ap()
        t["d_est"] = nc.dram_tensor("d_est", [P, NT, N], F8, kind="ExternalOutput").ap()
        t["d_recip"] = nc.dram_tensor("d_recip", [P, N], F32, kind="ExternalOutput").ap()
        t["d_outt"] = nc.dram_tensor("d_outt", [P, NH, N], F16, kind="ExternalOutput").ap()
    t["x"] = nc.dram_tensor("x", [IMGS, NH, P, N], F16, kind="ExternalInput").ap()
    t["wqk16"] = nc.dram_tensor("wqk16", [NH, P, 512], F16, kind="ExternalInput").ap()
    t["wv8"] = nc.dram_tensor("wv8", [NH, P, C], F8, kind="ExternalInput").ap()
    t["woT"] = nc.dram_tensor("woT", [NH, P, C], F16, kind="ExternalInput").ap()
    t["bqk"] = nc.dram_tensor("bqk", [4, P], F32, kind="ExternalInput").ap()
    t["gnw"] = nc.dram_tensor("gnw", [NH, P], F32, kind="ExternalInput").ap()
    t["gnb"] = nc.dram_tensor("gnb", [NH, P], F32, kind="ExternalInput").ap()
    t["fbv"] = nc.dram_tensor("fbv", [NH, P], F32, kind="ExternalInput").ap()
    t["ind"] = nc.dram_tensor("ind", [NH, P, GROUPS], F32, kind="ExternalInput").ap()
    t["indT"] = nc.dram_tensor("indT", [GROUPS, NH, P], F32, kind="ExternalInput").ap()
    t["out"] = nc.dram_tensor("out", [IMGS, NH, P, N], F16, kind="ExternalOutput").ap()
    with tile.TileContext(nc) as tc:
        with ExitStack() as ctx:
            _emit(ctx, tc, t, reps=reps, unroll=unroll)
    nc.compile()
    return nc


def _host_inputs(x, gn_w, gn_b, qkv_w, qkv_b, out_w, out_b):
    """Build the per-core input maps (host-side weight prep)."""
    x = np.asarray(x, dtype=np.float32).reshape(B, C, N)
    gn_w = np.asarray(gn_w, dtype=np.float32)
    gn_b = np.asarray(gn_b, dtype=np.float32)
    qkv_w = np.asarray(qkv_w, dtype=np.float32)
    qkv_b = np.asarray(qkv_b, dtype=np.float32)
    out_w = np.asarray(out_w, dtype=np.float32)
    out_b = np.asarray(out_b, dtype=np.float32)

    wqk16 = np.ascontiguousarray(qkv_w[:512].T).reshape(NH, P, 512).astype(np.float16)
    wv8 = np.ascontiguousarray(qkv_w[512:].T).reshape(NH, P, C).astype(
        ml_dtypes.float8_e4m3)
    woT = np.ascontiguousarray(out_w.T).reshape(NH, P, C).astype(np.float16)
    bqk = qkv_b[:512].reshape(4, P).astype(np.float32)

    fb = (out_w @ qkv_b[512:] + out_b).astype(np.float32)
    fbv = fb.reshape(NH, P)
    gnw = gn_w.reshape(NH, P)
    gnb = gn_b.reshape(NH, P)

    # local-group indicators (4 groups per 128-channel half, identical per half)
    ind = np.zeros((NH, P, GROUPS), np.float32)
    indT = np.zeros((GROUPS, NH, P), np.float32)
    cpg = C // GROUPS  # channels per group = 32
    for h in range(NH):
        for p in range(P):
            gl = p // cpg
            ind[h, p, gl] = 1.0 / cpg
            indT[gl, h, p] = 1.0

    shared = dict(wqk16=wqk16, wv8=wv8, woT=woT, bqk=bqk,
                  gnw=gnw, gnb=gnb, fbv=fbv, ind=ind, indT=indT)
    x16 = x.astype(np.float16)
    in_maps = []
    for core in range(N_CORES):
        xs = x16[core * IMGS:(core + 1) * IMGS].reshape(IMGS, NH, P, N)
        in_maps.append(dict(shared, x=np.ascontiguousarray(xs)))
    return in_maps


_NC_CACHE = {}


def _get_nc(reps: int = 1, unroll: int | None = None):
    if unroll is None:
        unroll = 4 if reps % 4 == 1 and reps > 1 else 1
    key = (reps, unroll)
    if key not in _NC_CACHE:
        _NC_CACHE[key] = _build(reps=reps, unroll=unroll)
    return _NC_CACHE[key]


def kernel(x, gn_w, gn_b, qkv_w, qkv_b, out_w, out_b, _reps=1, _unroll=None):
    nc = _get_nc(_reps, _unroll)
    in_maps = _host_inputs(x, gn_w, gn_b, qkv_w, qkv_b, out_w, out_b)
    res = run_bass_kernel_spmd(nc, in_maps, core_ids=list(range(N_CORES)))
    out = np.concatenate([
        np.asarray(r["out"], dtype=np.float32).reshape(IMGS, C, H, W)
        for r in res.results])
    kernel.last_results = res
    return out


# revision 15
# speedup vs baseline: 1.0258x; 1.0258x over previous
"""Trainium2 Bass kernel for GroupNorm -> self-attention -> proj + residual.

v5: fp16 matmuls for qkv/scores/proj; fp8 DoubleRow for the v-projection,
attn.V and colsum.  Colsum runs as a transient PSUM group at the end of
the scores stage so the big-PSUM pool gets 3 rotating buffers (6 banks).

Reference computation (per image, b=32 total, data-parallel over 8 cores):
    xn    = GroupNorm(x, 8 groups, affine)              [c=256, n=1024]
    q,k   = W_{q,k} @ xn + b_{q,k}      (fp16 matmul, fp16 drain)
    v     = W_v @ xn                    (fp8 DR matmul, fp8 drain)
    st    = k^T q                       (fp16, scores transposed [nk, nq])
    est   = exp(st/16 - C)              (C = 2.7 global shift, softmax-invariant)
    den   = colsum(est)                 (ones-matmul DR, transient PSUM)
    outT  = (v^T est) / den   [c, nq]   (fp8 DR)
    fin   = out_w @ outT + (out_w @ v_b + out_b) + xn   (fp16)

Design notes (HW-measured):
  - fp16 matmul (K=128, 512-wide) ~295ns HW; DR fp8 (K=256, 512-wide)
    ~255ns.  fp8 q/k for a DR scores matmul was measured at 2.4e-2 total
    rel err vs the 2e-2 gate (HW matches RTNE host emulation almost
    exactly; the fp8 est+v floor is ~1.2e-2) — so scores stay fp16.
  - est = exp(score/16 - C) in fp8e4 on ACT; C=2.7 keeps max est ~145,
    under the TRN e4m3 240 cap.
  - Softmax denominator folds into the outT drain; outT/out_w stay fp16.
  - Every PSUM tile has exactly ONE drain op (ACT or DVE) so PSUM buffers
    recycle fast; SBUF-only work (GroupNorm apply, residual prep) runs on
    the otherwise idle GPSIMD/Pool engine (which cannot touch PSUM).
  - Consecutive matmuls share their stationary operand where possible
    (loop orders chosen so lhsT repeats back-to-back).

Stages: S0 dma+GroupNorm, S1 qkv/vt, S2 scores+exp+colsum, S3 recip+attnv,
S4 proj+out-dma; emission is skewed so every engine always has runnable
work.  The reps timing loop runs `unroll` pipelined copies per iteration to
amortize For_i's per-iteration all-engine barrier and pipeline fill/drain.
"""

import numpy as np
import ml_dtypes
from contextlib import ExitStack

import concourse.bass as bass
import concourse.tile as tile
import concourse.mybir as mybir
from concourse import bacc
from concourse.bass import ts
from concourse.bass_utils import run_bass_kernel_spmd

P = 128
N_CORES = 8
B, C, H, W = 32, 256, 32, 32
N = H * W                      # 1024 pixels
IMGS = B // N_CORES            # 4 images per core
NH = C // P                    # 2 channel halves
NT = N // P                    # 8 pixel tiles
GROUPS = 8
EPS = 1e-5
F32 = mybir.dt.float32
F16 = mybir.dt.float16
F8 = mybir.dt.float8e4
AF = mybir.ActivationFunctionType
OP = mybir.AluOpType
DR = mybir.MatmulPerfMode.DoubleRow
CHUNK = 512                    # matmul moving free dim (one PSUM bank)
NCH = N // CHUNK               # 2 chunks
C_SHIFT = 2.7                  # global exp shift (softmax-invariant)

PHASE_OF = {}


class _phase:
    """Records which instructions each phase emits (for trace attribution)."""

    def __init__(self, nc, name):
        self.nc, self.name = nc, name

    def __enter__(self):
        self.before = set(self.nc.inst_map)
        return self

    def __exit__(self, *a):
        for n in set(self.nc.inst_map) - self.before:
            PHASE_OF[n] = self.name


class _State:
    """Per-image tiles passed between pipeline stages."""

    def __init__(self):
        self.xn16 = {}
        self.xn8 = {}
        self.xnfb = {}
        self.qk = {}
        self.vt = {}
        self.est = {}
        self.cs = {}
        self.recip = {}
        self.outt = {}


def _emit(ctx: ExitStack, tc: tile.TileContext, t: dict, reps: int = 1,
          unroll: int = 1):
    nc = tc.nc

    singles = ctx.enter_context(tc.tile_pool(name="singles", bufs=1))
    p_x = ctx.enter_context(tc.tile_pool(name="p_x", bufs=2))
    p_stats = ctx.enter_context(tc.tile_pool(name="p_stats", bufs=4))
    p_xn16 = ctx.enter_context(tc.tile_pool(name="p_xn16", bufs=3))
    p_xn8 = ctx.enter_context(tc.tile_pool(name="p_xn8", bufs=3))
    p_xnfb = ctx.enter_context(tc.tile_pool(name="p_xnfb", bufs=5))
    p_qk = ctx.enter_context(tc.tile_pool(name="p_qk", bufs=3))
    p_vt = ctx.enter_context(tc.tile_pool(name="p_vt", bufs=4))
    p_est = ctx.enter_context(tc.tile_pool(name="p_est", bufs=3))
    p_recip = ctx.enter_context(tc.tile_pool(name="p_recip", bufs=2))
    p_outt = ctx.enter_context(tc.tile_pool(name="p_outt", bufs=3))
    p_fin = ctx.enter_context(tc.tile_pool(name="p_fin", bufs=4))
    ps_big = ctx.enter_context(tc.tile_pool(name="ps_big", bufs=3, space="PSUM"))
    ps_sm = ctx.enter_context(tc.tile_pool(name="ps_sm", bufs=1, space="PSUM"))

    # ---- load constants / weights into SBUF once ----
    s_wqk16 = singles.tile([P, NH, 512], F16)
    nc.sync.dma_start(s_wqk16[:], t["wqk16"].rearrange("h p o -> p h o"))
    s_wv8 = singles.tile([P, NH, C], F8)
    nc.sync.dma_start(s_wv8[:], t["wv8"].rearrange("h p o -> p h o"))
    s_woT = singles.tile([P, NH, C], F16)
    nc.sync.dma_start(s_woT[:], t["woT"].rearrange("h p o -> p h o"))
    s_bqk = singles.tile([P, 4], F32)
    nc.sync.dma_start(s_bqk[:], t["bqk"].rearrange("j p -> p j"))
    s_gnw = singles.tile([P, NH], F32)
    nc.sync.dma_start(s_gnw[:], t["gnw"].rearrange("h p -> p h"))
    s_gnb = singles.tile([P, NH], F32)
    nc.sync.dma_start(s_gnb[:], t["gnb"].rearrange("h p -> p h"))
    s_fbv = singles.tile([P, NH], F32)      # out_w @ v_b + out_b (residual add)
    nc.sync.dma_start(s_fbv[:], t["fbv"].rearrange("h p -> p h"))
    s_ind = singles.tile([P, NH, GROUPS], F32)
    nc.sync.dma_start(s_ind[:], t["ind"].rearrange("h p g -> p h g"))
    s_indT = singles.tile([GROUPS, NH, P], F32)
    nc.sync.dma_start(s_indT[:], t["indT"])
    s_ones = singles.tile([P, NH, P], F8)
    nc.vector.memset(s_ones[:], 1.0)
    s_negC = singles.tile([P, 1], F32)
    nc.vector.memset(s_negC[:], -C_SHIFT)

    # PE warmup: dense dummy matmuls during the GroupNorm head so the HAM
    # clock-gate reaches 8/8 before the real matmuls start (HW-only effect).
    ps_w = ps_big.tile([P, N], F32, tag="big")
    for _ in range(10):
        nc.tensor.matmul(ps_w[:, 0:CHUNK], s_wqk16[:, 0, 0:P],
                         s_wqk16[:, 1, 0:CHUNK], start=True, stop=True)
    w_sink = p_stats.tile([1, 1], F32, tag="wsink")
    nc.vector.tensor_copy(w_sink[:], ps_w[0:1, 0:1])

    x_ap = t["x"]       # [IMGS, NH, P, N] f16
    out_ap = t["out"]   # [IMGS, NH, P, N] f16

    if reps > 1:
        # staggered_reset: no all-engine barrier at the back edge — the
        # body is split into 4 reset stages and consecutive iterations
        # overlap, so the 5-stage image pipeline never drains.
        loop = ctx.enter_context(  # noqa: F841 (timing loop)
            tc.For_i(0, reps // unroll, 1,
                     hint_engines=(mybir.EngineType.PE,),
                     staggered_reset=True))

    st = _State()

    def s0_gn(vimg):
        img = vimg % IMGS
        x_t = p_x.tile([P, NH, N], F16, tag="x")
        xn16 = st.xn16[vimg] = p_xn16.tile([P, NH, N], F16, tag="xn16",
                                           name=f"xn16_{vimg}")
        xn8 = st.xn8[vimg] = p_xn8.tile([P, NH, N], F8, tag="xn8",
                                        name=f"xn8_{vimg}")
        xnfb = st.xnfb[vimg] = p_xnfb.tile([P, NH, N], F16, tag="xnfb",
                                           name=f"xnfb_{vimg}")
        nc.sync.dma_start(x_t[:], x_ap[img].rearrange("h p n -> p h n"))
        for h in range(NH):
            # per-channel mean / E[x^2] via bn_stats (free dim cap 512)
            st6 = p_stats.tile([P, 2, 6], F32, tag="st6")
            xv = x_t[:, h].rearrange("p (s f) -> p s f", f=512)
            for s in range(2):
                nc.vector.bn_stats(out=st6[:, s, :], in_=xv[:, s, :])
            mv = p_stats.tile([P, 2], F32, tag="mv")
            nc.vector.bn_aggr(out=mv[:], in_=st6[:])
            mm = p_stats.tile([P, 2], F32, tag="mm")  # (mean, E[x^2])
            nc.vector.tensor_copy(mm[:, 0:1], mv[:, 0:1])
            nc.vector.tensor_tensor(mm[:, 1:2], mv[:, 0:1], mv[:, 0:1], OP.mult)
            nc.vector.tensor_tensor(mm[:, 1:2], mm[:, 1:2], mv[:, 1:2], OP.add)

            # this half's 4 group stats: [4, 2] = ind_h.T @ mm
            psg = ps_sm.tile([4, 2], F32, tag="sm")
            nc.tensor.matmul(psg[:], s_ind[:, h, :4], mm[:],
                             start=True, stop=True)
            grp = p_stats.tile([4, 2], F32, tag="grp")  # (mu, rstd)
            nc.vector.tensor_copy(grp[:, 0:1], psg[:, 0:1])
            nc.vector.tensor_copy(grp[:, 1:2], psg[:, 1:2])
            v = p_stats.tile([4, 3], F32, tag="musq")  # var+eps, s, t
            nc.vector.tensor_tensor(v[:, 1:2], grp[:, 0:1], grp[:, 0:1], OP.mult)
            nc.vector.tensor_tensor(v[:, 0:1], grp[:, 1:2], v[:, 1:2], OP.subtract)
            nc.vector.tensor_scalar(out=v[:, 0:1], in0=v[:, 0:1], scalar1=EPS,
                                    scalar2=None, op0=OP.add)
            # rstd = 1/sqrt(v) by Newton on sqrt from s0=1 (group var ~ 1),
            # all on DVE — keeps ACT's table set pinned to exp.
            nc.vector.tensor_scalar(out=v[:, 1:2], in0=v[:, 0:1], scalar1=1.0,
                                    scalar2=0.5, op0=OP.add, op1=OP.mult)
            for _ in range(2):
                nc.vector.reciprocal(v[:, 2:3], v[:, 1:2])
                nc.vector.tensor_tensor(v[:, 2:3], v[:, 0:1], v[:, 2:3], OP.mult)
                nc.vector.tensor_tensor(v[:, 1:2], v[:, 1:2], v[:, 2:3], OP.add)
                nc.vector.tensor_scalar(out=v[:, 1:2], in0=v[:, 1:2],
                                        scalar1=0.5, scalar2=None, op0=OP.mult)
            nc.vector.reciprocal(grp[:, 1:2], v[:, 1:2])

            # broadcast 4 group (mu, rstd) to this half's 128 channels
            psb = ps_sm.tile([P, 2], F32, tag="sm")
            nc.tensor.matmul(psb[:], s_indT[:4, h, :], grp[:],
                             start=True, stop=True)
            ab = p_stats.tile([P, 2], F32, tag="ab")  # a, b
            a = ab[:, 0:1]
            nc.vector.tensor_tensor(a, psb[:, 1:2], s_gnw[:, h:h + 1], OP.mult)
            mua = ab[:, 1:2]
            nc.vector.tensor_tensor(mua, psb[:, 0:1], a, OP.mult)
            nc.vector.tensor_tensor(mua, s_gnb[:, h:h + 1], mua, OP.subtract)

            # xn16 = f16(x*a+b); xn8 = f8 same; xnfb = f16(xn16 + fb)
            # (Pool: SBUF-only ops; Pool's fp8 rounding verified == DVE/ACT)
            nc.gpsimd.tensor_scalar(out=xn16[:, h], in0=x_t[:, h],
                                    scalar1=ab[:, 0:1], scalar2=ab[:, 1:2],
                                    op0=OP.mult, op1=OP.add)
            nc.gpsimd.tensor_scalar(out=xn8[:, h], in0=x_t[:, h],
                                    scalar1=ab[:, 0:1], scalar2=ab[:, 1:2],
                                    op0=OP.mult, op1=OP.add)
            nc.gpsimd.tensor_scalar(out=xnfb[:, h], in0=xn16[:, h],
                                    scalar1=s_fbv[:, h:h + 1], scalar2=None,
                                    op0=OP.add)

    def s1_qkv(vimg):
        xn16, xn8 = st.xn16[vimg], st.xn8[vimg]
        # fp16 q,k (biased in the drain): slots (q0,q1,k0,k1).  Scores must
        # stay fp16: fp8 q/k measured 2.0-2.4e-2 total rel err vs the 2e-2
        # gate (fp8 est+v alone already costs ~1.2e-2).
        qk = st.qk[vimg] = p_qk.tile([P, 4, N], F16, tag="qk",
                                     name=f"qk_{vimg}")
        for j in range(4):
            ps = ps_big.tile([P, N], F32, tag="big")
            for h in range(NH):       # h outer: consecutive MMs share lhsT
                for ch in range(NCH):
                    nc.tensor.matmul(ps[:, ts(ch, CHUNK)],
                                     s_wqk16[:, h, ts(j, P)],
                                     xn16[:, h, ts(ch, CHUNK)],
                                     start=(h == 0), stop=(h == NH - 1))
            if j < 2:
                nc.scalar.activation(out=qk[:, j], in_=ps[:], func=AF.Identity,
                                     bias=s_bqk[:, j:j + 1])
            else:
                nc.vector.tensor_scalar(out=qk[:, j], in0=ps[:],
                                        scalar1=s_bqk[:, j:j + 1], scalar2=None,
                                        op0=OP.add)

        # vT in [n, c] layout via fp8 DR (K=256 over the half pair)
        vt = st.vt[vimg] = p_vt.tile([P, NT, C], F8, tag="vt",
                                     name=f"vt_{vimg}")
        for tg in range(2):
            ps = ps_big.tile([P, N], F32, tag="big")
            for tl in range(4):
                tt = 4 * tg + tl
                nc.tensor.matmul(ps[:, ts(tl, C)],
                                 xn8[:, 0:NH, ts(tt, P)], s_wv8[:, 0:NH, :],
                                 start=True, stop=True, perf_mode=DR)
            nc.scalar.activation(out=vt[:, 4 * tg:4 * tg + 4], in_=ps[:],
                                 func=AF.Identity)

    def s2_scores(vimg):
        qk = st.qk[vimg]
        est = st.est[vimg] = p_est.tile([P, NT, N], F8, tag="est",
                                        name=f"est_{vimg}")
        for tt in range(NT):
            ps = ps_big.tile([P, N], F32, tag="big")
            for h in range(NH):       # h outer: consecutive MMs share lhsT
                for ch in range(NCH):
                    nc.tensor.matmul(ps[:, ts(ch, CHUNK)],
                                     qk[:, 2 + h, ts(tt, P)],
                                     qk[:, 0 + h, ts(ch, CHUNK)],
                                     start=(h == 0), stop=(h == NH - 1))
            nc.scalar.activation(out=est[:, tt], in_=ps[:], func=AF.Exp,
                                 bias=s_negC[:], scale=1.0 / 16.0)
        # colsum of est (ones-matmul DR, one shared stationary), transient
        # PSUM group drained by the recip in s3
        cs = st.cs[vimg] = ps_big.tile([P, N], F32, tag="big",
                                       name=f"cs_{vimg}")
        for i in range(NT // 2):
            for ch in range(NCH):
                nc.tensor.matmul(cs[:, ts(ch, CHUNK)], s_ones[:],
                                 est[:, 2 * i:2 * i + 2, ts(ch, CHUNK)],
                                 start=(i == 0), stop=(i == NT // 2 - 1),
                                 perf_mode=DR)

    def s3_attnv(vimg):
        vt, est, cs = st.vt[vimg], st.est[vimg], st.cs[vimg]
        recip = st.recip[vimg] = p_recip.tile([P, N], F32, tag="recip",
                                              name=f"recip_{vimg}")
        nc.vector.reciprocal(recip[:], cs[:])
        outt = st.outt[vimg] = p_outt.tile([P, NH, N], F16, tag="outt",
                                           name=f"outt_{vimg}")
        for m in range(NH):
            ps = ps_big.tile([P, N], F32, tag="big")
            for i in range(NT // 2):
                for ch in range(NCH):
                    nc.tensor.matmul(ps[:, ts(ch, CHUNK)],
                                     vt[:, 2 * i:2 * i + 2, ts(m, P)],
                                     est[:, 2 * i:2 * i + 2, ts(ch, CHUNK)],
                                     start=(i == 0), stop=(i == NT // 2 - 1),
                                     perf_mode=DR)
            # normalize during copyback (recip commutes with the c-contraction)
            nc.vector.tensor_tensor(outt[:, m], ps[:], recip[:], OP.mult)

    def s4_proj(vimg):
        img = vimg % IMGS
        outt, xnfb = st.outt[vimg], st.xnfb[vimg]
        for m in range(NH):
            ps = ps_big.tile([P, N], F32, tag="big")
            for h in range(NH):       # h outer: consecutive MMs share lhsT
                for ch in range(NCH):
                    nc.tensor.matmul(ps[:, ts(ch, CHUNK)],
                                     s_woT[:, h, ts(m, P)],
                                     outt[:, h, ts(ch, CHUNK)],
                                     start=(h == 0), stop=(h == NH - 1))
            fin = p_fin.tile([P, N], F16, tag="fin")
            nc.vector.tensor_tensor(fin[:], ps[:], xnfb[:, m], OP.add)
            nc.sync.dma_start(out_ap[img, m], fin[:])

    def _tap(vimg, stage_i):
        if vimg != 0 or "d_xn16" not in t:
            return
        if stage_i == 0:
            nc.sync.dma_start(t["d_xn16"], st.xn16[0][:])
            nc.sync.dma_start(t["d_xnfb"], st.xnfb[0][:])
        elif stage_i == 1:
            nc.sync.dma_start(t["d_qk"], st.qk[0][:])
            nc.sync.dma_start(t["d_vt"], st.vt[0][:])
        elif stage_i == 2:
            nc.sync.dma_start(t["d_est"], st.est[0][:])
        elif stage_i == 3:
            nc.sync.dma_start(t["d_recip"], st.recip[0][:])
            nc.sync.dma_start(t["d_outt"], st.outt[0][:])

    stages = [s0_gn, s1_qkv, s2_scores, s3_attnv, s4_proj]
    names = ["gn", "qkv", "scores", "attnv", "proj"]
    NS = len(stages)
    VIMGS = IMGS * unroll
    for t_step in range(VIMGS + NS - 1):
        for s in range(NS - 1, -1, -1):     # older images' later stages first
            vimg = t_step - s
            if 0 <= vimg < VIMGS:
                with _phase(nc, names[s]):
                    stages[s](vimg)
                    _tap(vimg, s)


def _build(reps: int = 1, unroll: int = 1, taps: bool = False):
    nc = bacc.Bacc("TRN2", debug=False, num_devices=N_CORES)
    t = {}
    if taps:
        t["d_xn16"] = nc.dram_tensor("d_xn16", [P, NH, N], F16, kind="ExternalOutput").ap()
        t["d_xnfb"] = nc.dram_tensor("d_xnfb", [P, NH, N], F16, kind="ExternalOutput").ap()
        t["d_qk"] = nc.dram_tensor("d_qk", [P, 4, N], F16, kind="ExternalOutput").ap()
        t["d_vt"] = nc.dram_tensor("d_vt", [P, NT, C], F8, kind="ExternalOutput").ap()
        t["d_est"] = nc.dram_tensor("d_est", [P, NT, N], F8, kind="ExternalOutput").ap()
        t["d_recip"] = nc.dram_tensor("d_recip", [P, N], F32, kind="ExternalOutput").ap()
        t["d_outt"] = nc.dram_tensor("d_outt", [P, NH, N], F16, kind="ExternalOutput").ap()
    t["x"] = nc.dram_tensor("x", [IMGS, NH, P, N], F16, kind="ExternalInput").ap()
    t["wqk16"] = nc.dram_tensor("wqk16", [NH, P, 512], F16, kind="ExternalInput").ap()
    t["wv8"] = nc.dram_tensor("wv8", [NH, P, C], F8, kind="ExternalInput").ap()
    t["woT"] = nc.dram_tensor("woT", [NH, P, C], F16, kind="ExternalInput").ap()
    t["bqk"] = nc.dram_tensor("bqk", [4, P], F32, kind="ExternalInput").ap()
    t["gnw"] = nc.dram_tensor("gnw", [NH, P], F32, kind="ExternalInput").ap()
    t["gnb"] = nc.dram_tensor("gnb", [NH, P], F32, kind="ExternalInput").ap()
    t["fbv"] = nc.dram_tensor("fbv", [NH, P], F32, kind="ExternalInput").ap()
    t["ind"] = nc.dram_tensor("ind", [NH, P, GROUPS], F32, kind="ExternalInput").ap()
    t["indT"] = nc.dram_tensor("indT", [GROUPS, NH, P], F32, kind="ExternalInput").ap()
    t["out"] = nc.dram_tensor("out", [IMGS, NH, P, N], F16, kind="ExternalOutput").ap()
    with tile.TileContext(nc) as tc:
        with ExitStack() as ctx:
            _emit(ctx, tc, t, reps=reps, unroll=unroll)
    nc.compile()
    return nc


def _host_inputs(x, gn_w, gn_b, qkv_w, qkv_b, out_w, out_b):
    """Build the per-core input maps (host-side weight prep)."""
    x = np.asarray(x, dtype=np.float32).reshape(B, C, N)
    gn_w = np.asarray(gn_w, dtype=np.float32)
    gn_b = np.asarray(gn_b, dtype=np.float32)
    qkv_w = np.asarray(qkv_w, dtype=np.float32)
    qkv_b = np.asarray(qkv_b, dtype=np.float32)
    out_w = np.asarray(out_w, dtype=np.float32)
    out_b = np.asarray(out_b, dtype=np.float32)

    wqk16 = np.ascontiguousarray(qkv_w[:512].T).reshape(NH, P, 512).astype(np.float16)
    wv8 = np.ascontiguousarray(qkv_w[512:].T).reshape(NH, P, C).astype(
        ml_dtypes.float8_e4m3)
    woT = np.ascontiguousarray(out_w.T).reshape(NH, P, C).astype(np.float16)
    bqk = qkv_b[:512].reshape(4, P).astype(np.float32)

    fb = (out_w @ qkv_b[512:] + out_b).astype(np.float32)
    fbv = fb.reshape(NH, P)
    gnw = gn_w.reshape(NH, P)
    gnb = gn_b.reshape(NH, P)

    # local-group indicators (4 groups per 128-channel half, identical per half)
    ind = np.zeros((NH, P, GROUPS), np.float32)
    indT = np.zeros((GROUPS, NH, P), np.float32)
    cpg = C // GROUPS  # channels per group = 32
    for h in range(NH):
        for p in range(P):
            gl = p // cpg
            ind[h, p, gl] = 1.0 / cpg
            indT[gl, h, p] = 1.0

    shared = dict(wqk16=wqk16, wv8=wv8, woT=woT, bqk=bqk,
                  gnw=gnw, gnb=gnb, fbv=fbv, ind=ind, indT=indT)
    x16 = x.astype(np.float16)
    in_maps = []
    for core in range(N_CORES):
        xs = x16[core * IMGS:(core + 1) * IMGS].reshape(IMGS, NH, P, N)
        in_maps.append(dict(shared, x=np.ascontiguousarray(xs)))
    return in_maps


_NC_CACHE = {}


def _get_nc(reps: int = 1, unroll: int | None = None):
    if unroll is None:
        unroll = 8 if reps % 8 == 1 and reps > 1 else (
            4 if reps % 4 == 1 and reps > 1 else 1)
    key = (reps, unroll)
    if key not in _NC_CACHE:
        _NC_CACHE[key] = _build(reps=reps, unroll=unroll)
    return _NC_CACHE[key]


def kernel(x, gn_w, gn_b, qkv_w, qkv_b, out_w, out_b, _reps=1, _unroll=None):
    nc = _get_nc(_reps, _unroll)
    in_maps = _host_inputs(x, gn_w, gn_b, qkv_w, qkv_b, out_w, out_b)
    res = run_bass_kernel_spmd(nc, in_maps, core_ids=list(range(N_CORES)))
    out = np.concatenate([
        np.asarray(r["out"], dtype=np.float32).reshape(IMGS, C, H, W)
        for r in res.results])
    kernel.last_results = res
    return out
